# revision 1
# baseline (speedup 1.0000x reference)
"""Chamfer distance loss kernel for Trainium2 (8 NeuronCores, SPMD).

Problem: B=4 batches, N=M=8192 points, D=3.  loss = sum over batches of
  sum_i min_j ||c1_i - c2_j||^2  +  sum_j min_i ||c2_j - c1_i||^2

Sharding: the 4 batches x 2 directions give exactly 8 independent
(A-cloud, B-cloud) brute-force nearest-neighbor tasks - one per core.
No collectives needed.

Shipped design (build_nc2): direct (a-b)^2 on ScalarE + VectorE.
B-side coords are replicated across all 128 partitions once (a single
stride-0 broadcast DMA); then each 128-point row-tile of A costs six
big [128, 8192] instructions:
  ACT Square(xB + (-xA)) / Square(yB + (-yA)) / Square(zB + (-zA))
  (per-partition bias = that partition's A coordinate), two DVE adds,
  and one DVE min-reduce -> mins[:, t].  The [128, 64] per-point minima
  go back to the host, which does the final (tiny) sum in float64.
Exact fp32 distance math, no |a|^2+|b|^2-2ab cancellation -- measured
4e-8 relative error vs the fp32 reference.

An alternative TensorE implementation (build_nc: K=24 exact-bf16-split
feature matmul producing the full distance matrix in PSUM + DVE
min-reduce) is kept below for reference; it is numerically equally good
but uses ~6x more instructions, which dominates cost in this axon
environment (per-instruction overhead >> cost-model time).

Toolchain notes: walrus here accepts at most ONE sync-wait command per
instruction (and none on custom ISA ops), while Tile emits fused
multi-waits -- _split_waits() hoists extras into standalone event-
semaphore instructions.  _strip_self_waits() removes same-engine waits
(guaranteed by program order) since each semaphore wait costs ~10us in
this environment.
"""

import numpy as np

try:
    import concourse.bass as bass  # noqa: F401
except ImportError:  # harness may run with a bare sys.path
    import sys

    for p in ("/root/.axon_site/_ro/trn_rl_repo", "/opt/trn_rl_repo", "/opt/pypackages"):
        if p not in sys.path:
            sys.path.append(p)
    import concourse.bass as bass  # noqa: F401

import ml_dtypes

B, N, M, D = 4, 8192, 8192, 3
KFEAT = 24
NCORES = 8
PT = 128          # A points per row-tile (PSUM partitions)
BLK = 512         # B points per matmul (one fp32 PSUM bank)
GROUP_BLKS = 4    # matmul banks per vector reduce ([128, 2048])

_BF16 = ml_dtypes.bfloat16


def _split3(v):
    """Exact 3-way bf16 split of fp32: v == vh + vl + vll (8+8+8 mantissa)."""
    vh = v.astype(_BF16).astype(np.float32)
    r = v - vh
    vl = r.astype(_BF16).astype(np.float32)
    vll = (r - vl).astype(_BF16).astype(np.float32)
    return vh, vl, vll


def _features(A, Bc):
    """Build the K=24 augmented feature matrices.

    A: [n,3] row-side cloud, Bc: [m,3] column-side cloud.
    Returns FA [24,n] bf16, FB [24,m] bf16 with
      FA[:,i] . FB[:,j] ~= ||A_i - B_j||^2  (fp32-accurate)
    """
    A = np.asarray(A, np.float32)
    Bc = np.asarray(Bc, np.float32)
    sqA = (A * A).sum(-1, dtype=np.float32)
    sqB = (Bc * Bc).sum(-1, dtype=np.float32)
    FA, FB = [], []
    for k in range(3):
        ah, al, all_ = _split3(A[:, k])
        bh, bl, bll = _split3(Bc[:, k])
        # kept products: hh, hl, lh, ll, h*ll, ll*h  (each exact in fp32)
        FA += [ah, ah, al, al, ah, all_]
        FB += [-2 * bh, -2 * bl, -2 * bh, -2 * bl, -2 * bll, -2 * bh]
    a1, a2, a3 = _split3(sqA)
    ones_m = np.ones_like(sqB)
    FA += [a1, a2, a3]
    FB += [ones_m, ones_m, ones_m]
    b1, b2, b3 = _split3(sqB)
    ones_n = np.ones_like(sqA)
    FA += [ones_n, ones_n, ones_n]
    FB += [b1, b2, b3]
    fa = np.stack(FA, 0).astype(_BF16)
    fb = np.stack(FB, 0).astype(_BF16)
    return fa, fb


import re as _re

_SELF_WAIT_RE = _re.compile(r"^(Pool|Activation|PE|DVE|SP)(_sequencer)?_\d+$")


def _strip_self_waits(nc):
    """Remove semaphore waits where an instruction waits on its OWN engine's
    proc semaphore.  Engines execute their instruction streams in order with
    in-order data completion (DVE/ACT drain between ops; PE matmul ends are
    pc-monotone), so these waits are redundant — and sem waits are extremely
    expensive (~10us) in this environment.  Cross-engine and DMA-proc waits
    are kept."""
    for f in nc.m.functions:
        for bb in f.blocks:
            for ins in bb.instructions:
                si = ins.sync_info
                if not si or not si.on_wait:
                    continue
                eng = str(ins.engine.value) if hasattr(ins.engine, "value") else str(ins.engine)
                kept = []
                for w in si.on_wait:
                    m = _SELF_WAIT_RE.match(w.ant_name or "")
                    if m and m.group(1) == eng:
                        continue
                    kept.append(w)
                if len(kept) != len(si.on_wait):
                    ins.sync_info = mybir_mod().SyncInfo(
                        on_wait=kept, on_update=list(si.on_update)
                    )
    return nc


def mybir_mod():
    from concourse import mybir

    return mybir


def _split_waits(nc, max_waits=1):
    """Walrus in this toolchain accepts at most one sync-wait command per
    instruction; Tile fuses several.  Hoist extra waits into standalone
    event-semaphore instructions right before the owner (same engine, so
    program order preserves semantics)."""
    from concourse import mybir

    for f in nc.m.functions:
        for bb in f.blocks:
            new_insts = []
            for ins in bb.instructions:
                si = ins.sync_info
                waits = list(si.on_wait) if si and si.on_wait else []
                # custom bass_isa instructions (e.g. PartitionBroadcast) can't
                # carry sync waits through walrus's visitInstISA at all
                lim = 0 if "bass_isa" in type(ins).__module__ else max_waits
                if len(waits) > lim:
                    extra, keep = (waits, []) if lim == 0 else (waits[:-lim], waits[-lim:])
                    for k, w in enumerate(extra):
                        ev = mybir.InstEventSemaphore(
                            name=f"{ins.name}-evw{k}", ins=[], outs=[]
                        )
                        ev.engine = ins.engine
                        ev.sync_info = mybir.SyncInfo(on_wait=[w], on_update=[])
                        new_insts.append(ev)
                    ins.sync_info = mybir.SyncInfo(
                        on_wait=keep, on_update=list(si.on_update)
                    )
                new_insts.append(ins)
            bb.instructions[:] = new_insts
    return nc


def build_nc(n_a=N, n_b=M, reps=1, group_blks=GROUP_BLKS, psum_bufs=2):
    """Build the per-core Bass program (SPMD: same program, per-core data)."""
    import concourse.tile as tile
    from concourse import mybir

    row_tiles = n_a // PT
    nblk = n_b // BLK
    ngroups = nblk // group_blks
    assert n_a % PT == 0 and n_b % (BLK * group_blks) == 0

    nc = bass.Bass("TRN2", target_bir_lowering=False, debug=False, num_devices=NCORES)
    # one packed input tensor -> a single input DMA (keeps the kernel-tail
    # drain within walrus's sync-wait-command limit)
    feat_d = nc.dram_tensor(
        "feat", [KFEAT, n_a + n_b], mybir.dt.bfloat16, kind="ExternalInput"
    )
    out_d = nc.dram_tensor("out", [PT, 1], mybir.dt.float32, kind="ExternalOutput")

    with tile.TileContext(nc) as tc:
        with (
            tc.tile_pool(name="const", bufs=1) as cpool,
            tc.tile_pool(name="psum", bufs=psum_bufs, space="PSUM") as ppool,
            tc.tile_pool(name="accum", bufs=1) as apool,
        ):
            feat = cpool.tile([KFEAT, n_a + n_b], mybir.dt.bfloat16)
            nc.sync.dma_start(feat[:], feat_d[:])
            af = feat[:, :n_a]
            bf = feat[:, n_a:]

            mins = apool.tile([PT, row_tiles * ngroups], mybir.dt.float32)
            m2 = apool.tile([PT, row_tiles], mybir.dt.float32)
            res = apool.tile([PT, 1], mybir.dt.float32)

            for _ in range(reps):
                for t in range(row_tiles):
                    lhsT = af[:, t * PT:(t + 1) * PT]
                    for g in range(ngroups):
                        ps = ppool.tile([PT, BLK * group_blks], mybir.dt.float32)
                        for q in range(group_blks):
                            j = g * group_blks + q
                            nc.tensor.matmul(
                                ps[:, q * BLK:(q + 1) * BLK],
                                lhsT,
                                bf[:, j * BLK:(j + 1) * BLK],
                                start=True,
                                stop=True,
                            )
                        nc.vector.tensor_reduce(
                            mins[:, t * ngroups + g: t * ngroups + g + 1],
                            ps[:],
                            axis=mybir.AxisListType.X,
                            op=mybir.AluOpType.min,
                        )
                nc.vector.tensor_reduce(
                    m2[:],
                    mins[:].rearrange("p (t g) -> p t g", g=ngroups),
                    axis=mybir.AxisListType.X,
                    op=mybir.AluOpType.min,
                )
                nc.vector.tensor_reduce(
                    res[:],
                    m2[:],
                    axis=mybir.AxisListType.X,
                    op=mybir.AluOpType.add,
                )
            nc.sync.dma_start(out_d[:], res[:])
    return _split_waits(_strip_self_waits(nc))


def build_nc2(n_a=N, n_b=M, reps=1, post=True):
    """v2: direct (a-b)^2 on ScalarE/VectorE, no PE/PSUM.

    Per 128-point row-tile of A (6 instructions):
      ACT: t1 = Square(xB_rep + (-xA))   [128, n_b]
      ACT: t2 = Square(yB_rep + (-yA))
      DVE: t1 += t2
      ACT: t2 = Square(zB_rep + (-zA))
      DVE: t1 += t2
      DVE: mins[:, t] = reduce_min(t1)
    B coords are replicated across partitions once via gpsimd
    partition_broadcast.  Exact fp32 distance math (no cancellation).
    """
    import concourse.tile as tile
    from concourse import mybir

    row_tiles = n_a // PT
    assert n_a % PT == 0

    nc = bass.Bass("TRN2", target_bir_lowering=False, debug=False, num_devices=NCORES)
    bc_d = nc.dram_tensor("bc", [1, 3 * n_b], mybir.dt.float32, kind="ExternalInput")
    ac_d = nc.dram_tensor("ac", [PT, 3 * row_tiles], mybir.dt.float32, kind="ExternalInput")
    out_d = nc.dram_tensor("out", [PT, row_tiles], mybir.dt.float32, kind="ExternalOutput")

    with tile.TileContext(nc) as tc:
        with tc.tile_pool(name="rep", bufs=1) as rpool:
            rep = rpool.tile([PT, 3 * n_b], mybir.dt.float32)
            # replicate the B coords across all 128 partitions in one DMA
            # (stride-0 partition dim on the DRAM side)
            nc.sync.dma_start(rep[:], bc_d[:].partition_broadcast(PT))
            with tc.tile_pool(name="work", bufs=1) as wpool:
                ac = wpool.tile([PT, 3 * row_tiles], mybir.dt.float32)
                nc.sync.dma_start(ac[:], ac_d[:])
                mins = wpool.tile([PT, row_tiles], mybir.dt.float32)
                t1 = wpool.tile([PT, n_b], mybir.dt.float32)
                t2 = wpool.tile([PT, n_b], mybir.dt.float32)
                xr = rep[:, 0:n_b]
                yr = rep[:, n_b:2 * n_b]
                zr = rep[:, 2 * n_b:3 * n_b]
                SQ = mybir.ActivationFunctionType.Square
                for _ in range(reps):
                    for t in range(row_tiles):
                        nxa = ac[:, t:t + 1]
                        nya = ac[:, row_tiles + t:row_tiles + t + 1]
                        nza = ac[:, 2 * row_tiles + t:2 * row_tiles + t + 1]
                        nc.scalar.activation(t1[:], xr, SQ, bias=nxa)
                        nc.scalar.activation(t2[:], yr, SQ, bias=nya)
                        nc.vector.tensor_tensor(t1[:], t1[:], t2[:], op=mybir.AluOpType.add)
                        nc.scalar.activation(t2[:], zr, SQ, bias=nza)
                        nc.vector.tensor_tensor(t1[:], t1[:], t2[:], op=mybir.AluOpType.add)
                        nc.vector.tensor_reduce(
                            mins[:, t:t + 1], t1[:],
                            axis=mybir.AxisListType.X, op=mybir.AluOpType.min,
                        )
                nc.sync.dma_start(out_d[:], mins[:])
    if post:
        return _split_waits(_strip_self_waits(nc))
    return nc


def make_in_maps2(cloud1, cloud2):
    """v2 inputs: bc = [xB||yB||zB] fp32, ac = negated A coords per row-tile."""
    in_maps = []
    for b in range(B):
        for A, Bc in ((cloud1[b], cloud2[b]), (cloud2[b], cloud1[b])):
            A = np.asarray(A, np.float32)
            Bc = np.asarray(Bc, np.float32)
            n_a, n_b = A.shape[0], Bc.shape[0]
            rt = n_a // PT
            bc = Bc.T.reshape(1, 3 * n_b).astype(np.float32)
            ac = np.concatenate(
                [-A[:, k].reshape(rt, PT).T for k in range(3)], axis=1
            ).astype(np.float32)
            in_maps.append({"bc": np.ascontiguousarray(bc), "ac": np.ascontiguousarray(ac)})
    return in_maps


def make_in_maps(cloud1, cloud2):
    """Per-core inputs: core 2b+0 handles (c1[b]->c2[b]), 2b+1 the reverse."""
    in_maps = []
    for b in range(B):
        for A, Bc in ((cloud1[b], cloud2[b]), (cloud2[b], cloud1[b])):
            fa, fb = _features(A, Bc)
            in_maps.append({"feat": np.concatenate([fa, fb], axis=1)})
    return in_maps


_NC_CACHE = {}


def kernel(cloud1, cloud2):
    from concourse.bass_utils import run_bass_kernel_spmd

    cloud1 = np.asarray(cloud1, np.float32)
    cloud2 = np.asarray(cloud2, np.float32)
    assert cloud1.shape == (B, N, D) and cloud2.shape == (B, M, D)

    if "nc2" not in _NC_CACHE:
        _NC_CACHE["nc2"] = build_nc2()
    nc = _NC_CACHE["nc2"]

    in_maps = make_in_maps2(cloud1, cloud2)
    results = run_bass_kernel_spmd(nc, in_maps, list(range(NCORES))).results
    total = 0.0
    for c in range(NCORES):
        total += float(results[c]["out"].astype(np.float64).sum())
    return np.array(total, dtype=np.float32)



# revision 2
# speedup vs baseline: 1.6372x; 1.6372x over previous
"""Chamfer distance loss kernel for Trainium2 (8 NeuronCores, SPMD).

Problem: B=4 batches, N=M=8192 points, D=3.
  loss = sum_b [ sum_i min_j ||c1_i - c2_j||^2 + sum_j min_i ||c2_j - c1_i||^2 ]

Design (all-DVE, shared distance matrix):
  Core c = (batch b = c//2, row-half h = c%2) computes the pairwise
  g_ij = |b_j|^2 - 2 a_i.b_j for its 4096 cloud1 rows x all 8192 cloud2
  points, and extracts BOTH chamfer directions from the same passes.
  Per 128-row tile (5 vector-engine instructions, each [128, 8192]):
    stt: u = (xb_rep * sx) + sqb_rep     (sx = -2*xa, per-partition scalar)
    stt: u = (yb_rep * sy) + u
    stt: u = (zb_rep * sz) + u           -> u = g_ij
    red: mins[:, t] = min_j u            (c1->c2; |a|^2 added on host)
    stt: macc = min(u + |a_i|^2, macc)   (running c2->c1 column mins;
                                          tile 0 uses op1=bypass as init)
  Host: c1->c2 = sum(mins) + sum(|a|^2); c2->c1 = elementwise min of the
  two half maccs, then min over the 128 partitions, then sum over j.

Why all-DVE: in this environment ACT instructions cost ~107us each at
[128,8192] and PE matmuls ~130us each, while DVE ops stream at
~1 elem/lane/cycle with negligible fixed cost.  scalar_tensor_tensor
(native InstTensorScalarPtr) fuses the per-partition scale multiply and
the tensor add/min into one pass, so the whole kernel is 161 DVE
instructions per core with no cross-engine semaphores in the body.
B-side coords+|b|^2 are replicated across partitions once via a single
stride-0 partition_broadcast DMA.

Toolchain notes: walrus accepts at most ONE sync-wait command per
instruction (none on custom ISA ops); _split_waits() hoists extras into
standalone event-semaphore instructions.  _strip_self_waits() removes
same-engine waits (guaranteed by program order).
"""

import numpy as np

try:
    import concourse.bass as bass  # noqa: F401
except ImportError:  # harness may run with a bare sys.path
    import sys

    for p in ("/root/.axon_site/_ro/trn_rl_repo", "/opt/trn_rl_repo", "/opt/pypackages"):
        if p not in sys.path:
            sys.path.append(p)
    import concourse.bass as bass  # noqa: F401

import re as _re

B, N, M, D = 4, 8192, 8192, 3
NCORES = 8
PT = 128          # rows per tile (SBUF partitions)
HT = N // 2       # 4096 cloud1 rows per core
RT = HT // PT     # 32 row tiles per core

_SELF_WAIT_RE = _re.compile(r"^(Pool|Activation|PE|DVE|SP)(_sequencer)?_\d+$")


def mybir_mod():
    from concourse import mybir

    return mybir


def _strip_self_waits(nc):
    """Remove semaphore waits where an instruction waits on its OWN engine's
    proc semaphore (engines execute in order with in-order data completion,
    so these are redundant and sem waits are expensive here)."""
    for f in nc.m.functions:
        for bb in f.blocks:
            for ins in bb.instructions:
                si = ins.sync_info
                if not si or not si.on_wait:
                    continue
                eng = str(ins.engine.value) if hasattr(ins.engine, "value") else str(ins.engine)
                kept = []
                for w in si.on_wait:
                    m = _SELF_WAIT_RE.match(w.ant_name or "")
                    if m and m.group(1) == eng:
                        continue
                    kept.append(w)
                if len(kept) != len(si.on_wait):
                    ins.sync_info = mybir_mod().SyncInfo(
                        on_wait=kept, on_update=list(si.on_update)
                    )
    return nc


def _split_waits(nc, max_waits=1):
    """Walrus accepts at most one sync-wait command per instruction (and none
    on custom bass_isa ops); hoist extras into standalone event-semaphore
    instructions right before the owner (same engine, program order)."""
    from concourse import mybir

    for f in nc.m.functions:
        for bb in f.blocks:
            new_insts = []
            for ins in bb.instructions:
                si = ins.sync_info
                waits = list(si.on_wait) if si and si.on_wait else []
                lim = 0 if "bass_isa" in type(ins).__module__ else max_waits
                if len(waits) > lim:
                    extra, keep = (waits, []) if lim == 0 else (waits[:-lim], waits[-lim:])
                    for k, w in enumerate(extra):
                        ev = mybir.InstEventSemaphore(
                            name=f"{ins.name}-evw{k}", ins=[], outs=[]
                        )
                        ev.engine = ins.engine
                        ev.sync_info = mybir.SyncInfo(on_wait=[w], on_update=[])
                        new_insts.append(ev)
                    ins.sync_info = mybir.SyncInfo(
                        on_wait=keep, on_update=list(si.on_update)
                    )
                new_insts.append(ins)
            bb.instructions[:] = new_insts
    return nc


def build_nc(reps=1, post=True, macc_out=True):
    """Per-core Bass program (SPMD: same program, per-core data)."""
    import concourse.tile as tile
    from concourse import mybir

    n_b = M
    rt = RT
    nc = bass.Bass("TRN2", target_bir_lowering=False, debug=False,
                   num_devices=NCORES)
    bc_d = nc.dram_tensor("bc", [1, 4 * n_b], mybir.dt.float32,
                          kind="ExternalInput")
    ac_d = nc.dram_tensor("ac", [PT, 4 * rt], mybir.dt.float32,
                          kind="ExternalInput")
    out_d = nc.dram_tensor("out", [PT, rt], mybir.dt.float32,
                           kind="ExternalOutput")
    if macc_out:
        macc_d = nc.dram_tensor("macc", [PT, n_b], mybir.dt.float32,
                                kind="ExternalOutput")

    MUL = mybir.AluOpType.mult
    ADD = mybir.AluOpType.add
    MIN = mybir.AluOpType.min
    BYP = mybir.AluOpType.bypass

    with tile.TileContext(nc) as tc:
        with tc.tile_pool(name="rep", bufs=1) as rpool:
            rep = rpool.tile([PT, 4 * n_b], mybir.dt.float32)
            nc.sync.dma_start(rep[:], bc_d[:].partition_broadcast(PT))
            with tc.tile_pool(name="work", bufs=1) as wpool:
                ac = wpool.tile([PT, 4 * rt], mybir.dt.float32)
                nc.sync.dma_start(ac[:], ac_d[:])
                mins = wpool.tile([PT, rt], mybir.dt.float32)
                macc = wpool.tile([PT, n_b], mybir.dt.float32)
                u = wpool.tile([PT, n_b], mybir.dt.float32)
                xr = rep[:, 0:n_b]
                yr = rep[:, n_b:2 * n_b]
                zr = rep[:, 2 * n_b:3 * n_b]
                qr = rep[:, 3 * n_b:4 * n_b]

                for _ in range(reps):
                    for t in range(rt):
                        sx = ac[:, t:t + 1]
                        sy = ac[:, rt + t:rt + t + 1]
                        sz = ac[:, 2 * rt + t:2 * rt + t + 1]
                        aq = ac[:, 3 * rt + t:3 * rt + t + 1]
                        nc.vector.scalar_tensor_tensor(
                            u[:], xr, sx, qr, op0=MUL, op1=ADD)
                        nc.vector.scalar_tensor_tensor(
                            u[:], yr, sy, u[:], op0=MUL, op1=ADD)
                        nc.vector.scalar_tensor_tensor(
                            u[:], zr, sz, u[:], op0=MUL, op1=ADD)
                        nc.vector.tensor_reduce(
                            mins[:, t:t + 1], u[:],
                            axis=mybir.AxisListType.X, op=MIN)
                        if t == 0:
                            # init: macc = u + |a|^2 (bypass ignores in1; u
                            # doubles as in1 so uninitialized macc is never
                            # an input)
                            nc.vector.scalar_tensor_tensor(
                                macc[:], u[:], aq, u[:], op0=ADD, op1=BYP)
                        else:
                            nc.vector.scalar_tensor_tensor(
                                macc[:], u[:], aq, macc[:], op0=ADD, op1=MIN)
                nc.sync.dma_start(out_d[:], mins[:])
                if macc_out:
                    nc.sync.dma_start(macc_d[:], macc[:])
    if post:
        return _split_waits(_strip_self_waits(nc))
    return nc


def make_in_maps(cloud1, cloud2):
    """Core 2b+h: batch b, cloud1 row-half h, vs all of cloud2."""
    in_maps = []
    sqa_half = []
    for b in range(B):
        A = np.asarray(cloud1[b], np.float32)
        Bc = np.asarray(cloud2[b], np.float32)
        sqb = (Bc * Bc).sum(-1).astype(np.float32)
        bc = np.ascontiguousarray(
            np.concatenate([Bc[:, 0], Bc[:, 1], Bc[:, 2], sqb]
                           ).reshape(1, 4 * M).astype(np.float32))
        sqa = (A * A).sum(-1).astype(np.float32)
        for h in range(2):
            Ah = A[h * HT:(h + 1) * HT]
            sqah = sqa[h * HT:(h + 1) * HT]
            ac = np.concatenate(
                [(-2.0 * Ah[:, k]).reshape(RT, PT).T for k in range(3)]
                + [sqah.reshape(RT, PT).T], axis=1).astype(np.float32)
            in_maps.append({"bc": bc, "ac": np.ascontiguousarray(ac)})
            sqa_half.append(float((Ah.astype(np.float64) ** 2).sum()))
    return in_maps, sqa_half


_NC_CACHE = {}


def kernel(cloud1, cloud2):
    from concourse.bass_utils import run_bass_kernel_spmd

    cloud1 = np.asarray(cloud1, np.float32)
    cloud2 = np.asarray(cloud2, np.float32)
    assert cloud1.shape == (B, N, D) and cloud2.shape == (B, M, D)

    if "nc" not in _NC_CACHE:
        _NC_CACHE["nc"] = build_nc()
    nc = _NC_CACHE["nc"]

    in_maps, sqa_half = make_in_maps(cloud1, cloud2)
    results = run_bass_kernel_spmd(nc, in_maps, list(range(NCORES))).results
    total = 0.0
    for b in range(B):
        r0, r1 = results[2 * b], results[2 * b + 1]
        total += float(r0["out"].astype(np.float64).sum()) + sqa_half[2 * b]
        total += float(r1["out"].astype(np.float64).sum()) + sqa_half[2 * b + 1]
        mc = np.minimum(r0["macc"], r1["macc"]).min(axis=0)
        total += float(mc.astype(np.float64).sum())
    return np.array(total, dtype=np.float32)
